# revision 1
# baseline (speedup 1.0000x reference)
"""GPT decoder (B=8,T=256,D=512,H=8,L=6,DFF=2048,V=50257) on 8 TRN2 NeuronCores.

Strategy:
- Layers: data-parallel over batch (core c owns batch c). bf16 matmuls with
  fp32 PSUM accumulation; fp32 residual stream, LN and softmax statistics.
- Vocab projection: tensor-parallel over vocab. Final activations are
  transposed locally, AllGathered across the 8 cores (bf16), and each core
  computes all 2048 tokens against its 6284-column shard of Wout.
- Host side only reshapes/casts/shards inputs and unshards the output.
"""
import math
import os

import numpy as np
import ml_dtypes

import concourse.bass as bass
import concourse.tile as tile
from concourse import bacc, mybir
from concourse import bass_utils
from concourse.masks import make_identity

F32 = mybir.dt.float32
BF16 = mybir.dt.bfloat16
I32 = mybir.dt.int32

D = 512
T = 256
H = 8
DK = 64
L = 6
DFF = 2048
V = 50257
B = 8
NCORES = 8
P = 128

VS = 6284           # per-core vocab shard (8 * 6284 = 50272 >= 50257)
VPAD = VS * NCORES
TT = 2              # token tiles per core (T / P)
KB = D // P         # 4 contraction chunks over D
FB = DFF // P       # 16 chunks over DFF
NTILES = [512] * (VS // 512) + ([VS % 512] if VS % 512 else [])  # vocab n-tiles

_CACHE: dict = {}


def _build_program():
    nc = bacc.Bacc("TRN2", target_bir_lowering=False, debug=False,
                   num_devices=NCORES)

    # ---- I/O declarations ------------------------------------------------
    idx_h = nc.dram_tensor("idx", [T, 1], I32, kind="ExternalInput")
    emb_h = nc.dram_tensor("emb", [V, D], BF16, kind="ExternalInput")
    posenc_h = nc.dram_tensor("posenc", [T, D], F32, kind="ExternalInput")
    mask_h = nc.dram_tensor("maskadd", [T, T], F32, kind="ExternalInput")
    # weights pre-tiled on host: partition dim second-to-innermost
    wqkv_h = nc.dram_tensor("wqkv", [L, 3, P, KB, D], BF16, kind="ExternalInput")
    wo_h = nc.dram_tensor("wo", [L, P, KB, D], BF16, kind="ExternalInput")
    w1_h = nc.dram_tensor("w1", [L, P, KB, DFF], BF16, kind="ExternalInput")
    w2_h = nc.dram_tensor("w2", [L, P, FB, D], BF16, kind="ExternalInput")
    b1t_h = nc.dram_tensor("b1t", [L, P, FB], F32, kind="ExternalInput")
    b2_h = nc.dram_tensor("b2", [L, D], F32, kind="ExternalInput")
    ln1g_h = nc.dram_tensor("ln1g", [L, D], F32, kind="ExternalInput")
    ln1b_h = nc.dram_tensor("ln1b", [L, D], F32, kind="ExternalInput")
    ln2g_h = nc.dram_tensor("ln2g", [L, D], F32, kind="ExternalInput")
    ln2b_h = nc.dram_tensor("ln2b", [L, D], F32, kind="ExternalInput")
    lnfg_h = nc.dram_tensor("lnfg", [D], F32, kind="ExternalInput")
    lnfb_h = nc.dram_tensor("lnfb", [D], F32, kind="ExternalInput")
    wout_h = nc.dram_tensor("wout", [P, KB, VS], BF16, kind="ExternalInput")
    bout_h = nc.dram_tensor("bout", [VS], F32, kind="ExternalInput")
    logits_h = nc.dram_tensor("logits", [B * T, VS], F32, kind="ExternalOutput")

    scale = 1.0 / math.sqrt(D)

    def bcast_row(dram_1d_ap, n):
        """DMA-broadcast a [n] DRAM row across all 128 partitions."""
        return bass.AP(tensor=dram_1d_ap.tensor, offset=dram_1d_ap.offset,
                       ap=[[0, P], [1, n]])

    with tile.TileContext(nc) as tc:
        from contextlib import ExitStack
        with ExitStack() as ctx:
            consts = ctx.enter_context(tc.tile_pool(name="consts", bufs=1))
            acts = ctx.enter_context(tc.tile_pool(name="acts", bufs=1))
            scr = ctx.enter_context(tc.tile_pool(name="scr", bufs=3))
            scr2 = ctx.enter_context(tc.tile_pool(name="scr2", bufs=2))
            psB = ctx.enter_context(tc.tile_pool(name="psB", bufs=2, space="PSUM"))
            psM = ctx.enter_context(tc.tile_pool(name="psM", bufs=4, space="PSUM"))
            psT = ctx.enter_context(tc.tile_pool(name="psT", bufs=2, space="PSUM"))
            dram = ctx.enter_context(tc.tile_pool(name="dram", bufs=1, space="DRAM"))

            # ---- constants ----
            ident = consts.tile([P, P], BF16)
            make_identity(nc, ident)
            eps_sb = consts.tile([P, 1], F32)
            nc.vector.memset(eps_sb, 1e-5)
            posenc_sb = consts.tile([P, TT, D], F32)
            nc.sync.dma_start(out=posenc_sb,
                              in_=posenc_h.ap().rearrange("(t p) d -> p t d", p=P))
            mask_sb = consts.tile([P, TT, T], F32)
            nc.sync.dma_start(out=mask_sb,
                              in_=mask_h.ap().rearrange("(t p) s -> p t s", p=P))
            lnfg_sb = consts.tile([P, D], F32)
            nc.sync.dma_start(out=lnfg_sb, in_=bcast_row(lnfg_h.ap(), D))
            lnfb_sb = consts.tile([P, D], F32)
            nc.sync.dma_start(out=lnfb_sb, in_=bcast_row(lnfb_h.ap(), D))

            # ---- persistent activations ----
            x = acts.tile([P, TT, D], F32)          # residual stream
            xn = acts.tile([P, TT, D], BF16)        # post-LN activations
            xnT = acts.tile([P, KB, T], BF16)       # transposed post-LN
            qt = acts.tile([P, KB, T], BF16)        # Q^T (head-major partitions)
            kt = acts.tile([P, KB, T], BF16)        # K^T
            vv = acts.tile([P, TT, D], BF16)        # V natural [t, h*dk]
            ot = acts.tile([P, KB, T], BF16)        # attn out^T
            ht = acts.tile([P, FB, T], BF16)        # FFN hidden^T

            # ---- embedding gather + positional encoding ----
            idx_sb = acts.tile([P, TT], I32)
            nc.sync.dma_start(out=idx_sb,
                              in_=idx_h.ap().rearrange("(t p) one -> p (t one)", p=P))
            for t in range(TT):
                emb_g = scr.tile([P, D], BF16, name="emb_g")
                nc.gpsimd.indirect_dma_start(
                    out=emb_g[:], out_offset=None,
                    in_=emb_h.ap(),
                    in_offset=bass.IndirectOffsetOnAxis(ap=idx_sb[:, t:t + 1], axis=0),
                )
                emb_f = scr.tile([P, D], F32, name="emb_f")
                nc.vector.tensor_copy(out=emb_f, in_=emb_g)
                nc.vector.tensor_add(out=x[:, t], in0=emb_f, in1=posenc_sb[:, t])

            def layernorm(g_rep, b_rep, out_bf):
                """LN over the residual stream x -> out_bf (bf16), fp32 stats."""
                for t in range(TT):
                    stats = scr.tile([P, 6], F32, name="ln_stats")
                    nc.vector.bn_stats(out=stats, in_=x[:, t])
                    mv = scr.tile([P, 2], F32, name="ln_mv")
                    nc.vector.bn_aggr(out=mv, in_=stats)
                    rstd = scr.tile([P, 1], F32, name="ln_rstd")
                    nc.scalar.activation(out=rstd, in_=mv[:, 1:2],
                                         func=mybir.ActivationFunctionType.Sqrt,
                                         bias=eps_sb, scale=1.0)
                    nc.vector.reciprocal(out=rstd, in_=rstd)
                    z = scr.tile([P, D], F32, name="ln_z")
                    nc.vector.tensor_scalar(out=z, in0=x[:, t],
                                            scalar1=mv[:, 0:1], scalar2=rstd,
                                            op0=mybir.AluOpType.subtract,
                                            op1=mybir.AluOpType.mult)
                    nc.vector.tensor_mul(out=z, in0=z, in1=g_rep)
                    nc.vector.tensor_add(out=out_bf[:, t], in0=z, in1=b_rep)

            def transpose_2x4(src_bf, dst):
                """[128, TT, D] token-major -> [128, KB, T] feature-major."""
                for t in range(TT):
                    for kb in range(KB):
                        tp = psT.tile([P, P], BF16, name="pst")
                        nc.tensor.transpose(out=tp[:],
                                            in_=src_bf[:, t, kb * P:(kb + 1) * P],
                                            identity=ident[:])
                        nc.vector.tensor_copy(out=dst[:, kb, t * P:(t + 1) * P],
                                              in_=tp[:])

            # ================= decoder layers =================
            with tc.tile_pool(name="wpool", bufs=2) as wp:
                for l in range(L):
                    # ---- stream this layer's weights ----
                    wqkv_t = wp.tile([P, 3, KB, D], BF16, name="wqkv_t")
                    for m in range(3):
                        nc.sync.dma_start(out=wqkv_t[:, m], in_=wqkv_h.ap()[l, m])
                    wo_t = wp.tile([P, KB, D], BF16, name="wo_t")
                    nc.sync.dma_start(out=wo_t, in_=wo_h.ap()[l])
                    w1_t = wp.tile([P, KB, DFF], BF16, name="w1_t")
                    nc.sync.dma_start(out=w1_t, in_=w1_h.ap()[l])
                    w2_t = wp.tile([P, FB, D], BF16, name="w2_t")
                    nc.sync.dma_start(out=w2_t, in_=w2_h.ap()[l])
                    b1_sb = wp.tile([P, FB], F32, name="b1_sb")
                    nc.sync.dma_start(out=b1_sb, in_=b1t_h.ap()[l])
                    g1 = wp.tile([P, D], F32, name="g1")
                    nc.sync.dma_start(out=g1, in_=bcast_row(ln1g_h.ap()[l], D))
                    bb1 = wp.tile([P, D], F32, name="bb1")
                    nc.sync.dma_start(out=bb1, in_=bcast_row(ln1b_h.ap()[l], D))
                    g2 = wp.tile([P, D], F32, name="g2")
                    nc.sync.dma_start(out=g2, in_=bcast_row(ln2g_h.ap()[l], D))
                    bb2 = wp.tile([P, D], F32, name="bb2")
                    nc.sync.dma_start(out=bb2, in_=bcast_row(ln2b_h.ap()[l], D))
                    b2r = wp.tile([P, D], F32, name="b2r")
                    nc.sync.dma_start(out=b2r, in_=bcast_row(b2_h.ap()[l], D))

                    # ---- LN1 + transpose ----
                    layernorm(g1, bb1, xn)
                    transpose_2x4(xn, xnT)

                    # ---- Q^T, K^T (head-pair-major), V natural ----
                    for m, dst in ((0, qt), (1, kt)):
                        for pair in range(KB):
                            ps = psM.tile([P, T], F32, name="psm")
                            for kb in range(KB):
                                nc.tensor.matmul(
                                    ps[:],
                                    wqkv_t[:, m, kb, pair * P:(pair + 1) * P],
                                    xnT[:, kb],
                                    start=(kb == 0), stop=(kb == KB - 1))
                            nc.vector.tensor_copy(out=dst[:, pair], in_=ps[:])
                    for t in range(TT):
                        ps = psB.tile([P, D], F32, name="psb")
                        for kb in range(KB):
                            nc.tensor.matmul(ps[:], xnT[:, kb, t * P:(t + 1) * P],
                                             wqkv_t[:, 2, kb],
                                             start=(kb == 0), stop=(kb == KB - 1))
                        nc.vector.tensor_copy(out=vv[:, t], in_=ps[:])

                    # ---- attention, one head at a time ----
                    for pair in range(KB):
                        ot_ps = psM.tile([P, T], F32, name="psm")
                        for sub in range(2):
                            h = pair * 2 + sub
                            off = sub * DK
                            at = scr2.tile([P, TT, T], BF16, name="at")
                            for tq in range(TT):
                                s_ps = psM.tile([P, T], F32, name="psm")
                                nc.tensor.matmul(
                                    s_ps[:],
                                    qt[off:off + DK, pair, tq * P:(tq + 1) * P],
                                    kt[off:off + DK, pair],
                                    start=True, stop=True)
                                sm = scr.tile([P, T], F32, name="sm")
                                nc.vector.tensor_add(out=sm, in0=s_ps,
                                                     in1=mask_sb[:, tq])
                                pexp = scr.tile([P, T], F32, name="pexp")
                                den = scr.tile([P, 1], F32, name="den")
                                nc.scalar.activation(
                                    out=pexp, in_=sm,
                                    func=mybir.ActivationFunctionType.Exp,
                                    scale=scale, accum_out=den)
                                nc.vector.reciprocal(out=den, in_=den)
                                a_bf = scr.tile([P, T], BF16, name="a_bf")
                                nc.vector.tensor_scalar_mul(out=a_bf, in0=pexp,
                                                            scalar1=den)
                                for tk in range(TT):
                                    tp = psT.tile([P, P], BF16, name="pst")
                                    nc.tensor.transpose(
                                        out=tp[:],
                                        in_=a_bf[:, tk * P:(tk + 1) * P],
                                        identity=ident[:])
                                    nc.vector.tensor_copy(
                                        out=at[:, tk, tq * P:(tq + 1) * P],
                                        in_=tp[:])
                            # O^T (this head) = V_h^T @ A^T
                            for tk in range(TT):
                                nc.tensor.matmul(
                                    ot_ps[off:off + DK, :],
                                    vv[:, tk, h * DK:(h + 1) * DK],
                                    at[:, tk],
                                    start=(tk == 0), stop=(tk == TT - 1))
                        nc.vector.tensor_copy(out=ot[:, pair], in_=ot_ps[:])

                    # ---- x += O @ Wo ----
                    for tq in range(TT):
                        ps = psB.tile([P, D], F32, name="psb")
                        for kb in range(KB):
                            nc.tensor.matmul(ps[:], ot[:, kb, tq * P:(tq + 1) * P],
                                             wo_t[:, kb],
                                             start=(kb == 0), stop=(kb == KB - 1))
                        nc.vector.tensor_add(out=x[:, tq], in0=x[:, tq], in1=ps[:])

                    # ---- LN2 + FFN ----
                    layernorm(g2, bb2, xn)
                    transpose_2x4(xn, xnT)
                    for fc in range(FB):
                        ps = psM.tile([P, T], F32, name="psm")
                        for kb in range(KB):
                            nc.tensor.matmul(ps[:],
                                             w1_t[:, kb, fc * P:(fc + 1) * P],
                                             xnT[:, kb],
                                             start=(kb == 0), stop=(kb == KB - 1))
                        nc.scalar.activation(out=ht[:, fc], in_=ps[:],
                                             func=mybir.ActivationFunctionType.Relu,
                                             bias=b1_sb[:, fc:fc + 1], scale=1.0)
                    for tq in range(TT):
                        ps = psB.tile([P, D], F32, name="psb")
                        for fc in range(FB):
                            nc.tensor.matmul(ps[:], ht[:, fc, tq * P:(tq + 1) * P],
                                             w2_t[:, fc],
                                             start=(fc == 0), stop=(fc == FB - 1))
                        nc.vector.tensor_add(out=x[:, tq], in0=x[:, tq], in1=ps[:])
                        nc.vector.tensor_add(out=x[:, tq], in0=x[:, tq], in1=b2r)

            # ================= final LN + all-gather =================
            layernorm(lnfg_sb, lnfb_sb, xn)
            transpose_2x4(xn, xnT)
            ag_in = dram.tile([D, T], BF16)
            ag_out = dram.tile([NCORES * D, T], BF16, addr_space="Shared")
            for kb in range(KB):
                nc.sync.dma_start(out=ag_in[kb * P:(kb + 1) * P, :], in_=xnT[:, kb])
            nc.gpsimd.collective_compute(
                "AllGather", mybir.AluOpType.bypass,
                replica_groups=[list(range(NCORES))],
                ins=[ag_in[:]], outs=[ag_out[:]])

            # ================= vocab projection =================
            with tc.tile_pool(name="vpool", bufs=1) as vp, \
                 tc.tile_pool(name="vstream", bufs=3) as vs, \
                 tc.tile_pool(name="vout", bufs=4) as vo:
                xg = vp.tile([P, KB, B * T], BF16)
                for b in range(B):
                    for kb in range(KB):
                        nc.sync.dma_start(
                            out=xg[:, kb, b * T:(b + 1) * T],
                            in_=ag_out[b * D + kb * P: b * D + (kb + 1) * P, :])
                n0 = 0
                for nsz in NTILES:
                    wout_t = vs.tile([P, KB, 512], BF16, name="wout_t")
                    nc.sync.dma_start(out=wout_t[:, :, :nsz],
                                      in_=wout_h.ap()[:, :, n0:n0 + nsz])
                    boutr = vs.tile([P, 512], F32, name="boutr")
                    nc.sync.dma_start(out=boutr[:, :nsz],
                                      in_=bcast_row(bout_h.ap()[n0:n0 + nsz], nsz))
                    for tq in range(B * T // P):
                        ps = psB.tile([P, 512], F32, name="psb")
                        for kb in range(KB):
                            nc.tensor.matmul(ps[:, :nsz],
                                             xg[:, kb, tq * P:(tq + 1) * P],
                                             wout_t[:, kb, :nsz],
                                             start=(kb == 0), stop=(kb == KB - 1))
                        lg = vo.tile([P, 512], F32, name="lg")
                        nc.vector.tensor_add(out=lg[:, :nsz], in0=ps[:, :nsz],
                                             in1=boutr[:, :nsz])
                        nc.sync.dma_start(
                            out=logits_h.ap()[tq * P:(tq + 1) * P, n0:n0 + nsz],
                            in_=lg[:, :nsz])
                    n0 += nsz

    nc.compile()
    return nc


def _prep_inputs(inputs):
    """Host-side shard/cast/layout. Returns per-core input maps."""
    f32 = np.float32
    bf16 = ml_dtypes.bfloat16

    idx = np.asarray(inputs["idx"])
    emb = np.asarray(inputs["emb"], f32)

    # positional encoding (input-independent constant)
    pos = np.arange(T, dtype=np.float64)[:, None]
    div = np.exp(np.arange(0, D, 2, dtype=np.float64) * (-math.log(10000.0) / D))
    pe = np.zeros((T, D), f32)
    pe[:, 0::2] = np.sin(pos * div).astype(f32)
    pe[:, 1::2] = np.cos(pos * div).astype(f32)

    maskadd = np.where(np.tril(np.ones((T, T), bool)), 0.0, -1e9).astype(f32)

    wq = np.asarray(inputs["Wq"], f32)  # [L, H, D, DK]
    wk = np.asarray(inputs["Wk"], f32)
    wv = np.asarray(inputs["Wv"], f32)
    # [L, 3, D, H*DK] -> pre-tiled [L, 3, P, KB, D]
    wqkv = np.stack([
        wq.transpose(0, 2, 1, 3).reshape(L, D, D),
        wk.transpose(0, 2, 1, 3).reshape(L, D, D),
        wv.transpose(0, 2, 1, 3).reshape(L, D, D),
    ], axis=1)
    wqkv_t = np.ascontiguousarray(
        wqkv.reshape(L, 3, KB, P, D).transpose(0, 1, 3, 2, 4)).astype(bf16)
    wo_t = np.ascontiguousarray(
        np.asarray(inputs["Wo"], f32).reshape(L, KB, P, D)
        .transpose(0, 2, 1, 3)).astype(bf16)
    w1_t = np.ascontiguousarray(
        np.asarray(inputs["W1"], f32).reshape(L, KB, P, DFF)
        .transpose(0, 2, 1, 3)).astype(bf16)
    w2_t = np.ascontiguousarray(
        np.asarray(inputs["W2"], f32).reshape(L, FB, P, D)
        .transpose(0, 2, 1, 3)).astype(bf16)
    b1t = np.ascontiguousarray(
        np.asarray(inputs["b1"], f32).reshape(L, FB, P).transpose(0, 2, 1))

    wout = np.asarray(inputs["Wout"], f32)
    bout = np.asarray(inputs["bout"], f32)
    wout_pad = np.zeros((D, VPAD), f32)
    wout_pad[:, :V] = wout
    bout_pad = np.zeros((VPAD,), f32)
    bout_pad[:V] = bout

    emb_bf = emb.astype(bf16)

    common = dict(
        emb=emb_bf, posenc=pe, maskadd=maskadd,
        wqkv=wqkv_t, wo=wo_t, w1=w1_t, w2=w2_t, b1t=b1t,
        b2=np.asarray(inputs["b2"], f32),
        ln1g=np.asarray(inputs["ln1_g"], f32), ln1b=np.asarray(inputs["ln1_b"], f32),
        ln2g=np.asarray(inputs["ln2_g"], f32), ln2b=np.asarray(inputs["ln2_b"], f32),
        lnfg=np.asarray(inputs["lnf_g"], f32), lnfb=np.asarray(inputs["lnf_b"], f32),
    )
    in_maps = []
    for c in range(NCORES):
        m = dict(common)
        m["idx"] = np.ascontiguousarray(idx[c].astype(np.int32).reshape(T, 1))
        ws = wout_pad[:, c * VS:(c + 1) * VS]
        m["wout"] = np.ascontiguousarray(
            ws.reshape(KB, P, VS).transpose(1, 0, 2)).astype(bf16)
        m["bout"] = np.ascontiguousarray(bout_pad[c * VS:(c + 1) * VS])
        in_maps.append(m)
    return in_maps


def _unshard(results):
    shards = [results[c]["logits"] for c in range(NCORES)]  # each [B*T, VS]
    full = np.concatenate(shards, axis=1)[:, :V]            # [B*T, V]
    return np.ascontiguousarray(full.reshape(B, T, V))


def kernel(**inputs):
    if "nc" not in _CACHE:
        _CACHE["nc"] = _build_program()
    nc = _CACHE["nc"]
    in_maps = _prep_inputs(inputs)

    if os.environ.get("KERNEL_USE_SIM"):
        from concourse.bass_interp import MultiCoreSim
        sim = MultiCoreSim(nc, num_cores=NCORES,
                           num_workers=int(os.environ.get("KERNEL_SIM_WORKERS", "8")))
        for c in range(NCORES):
            for name, val in in_maps[c].items():
                sim.cores[c].tensor(name)[:] = val
        sim.simulate()
        results = [
            {"logits": np.array(sim.cores[c].tensor("logits"))}
            for c in range(NCORES)
        ]
        return _unshard(results)

    res = bass_utils.run_bass_kernel_spmd(
        nc, in_maps, core_ids=list(range(NCORES)))
    return _unshard(res.results)



# revision 21
# speedup vs baseline: 1.2313x; 1.2313x over previous
"""GPT decoder (B=8,T=256,D=512,H=8,L=6,DFF=2048,V=50257) on 8 TRN2 NeuronCores.

Strategy (v2):
- Data-parallel over batch (core c owns batch c); vocab projection is
  tensor-parallel over vocab (per-core shard 6284, padded to 6400).
- LN gains/biases folded into weights host-side (exact algebra); on-chip LN
  only computes (x-mean)*rstd, with rstd = exp(-0.5*ln(var+eps)) so the
  whole kernel uses a single ACT table set (natural_log_exp_and_others).
- Residual stream lives in PSUM: Wo/W2/bias adds are matmul accumulations.
- Causal skip: the fully-masked (tq0, tk1) block is never computed; the
  causal mask is preloaded into PSUM via identity-matmul (no DVE adds).
- fp16 activations/weights (same PE throughput as bf16, 8x the mantissa).
- Vocab projection computed transposed (logits^T) so bout is a
  per-partition ACT bias; Wout tile stays stationary across 4 token-slot
  matmuls; output DMA'd as contiguous fp16 tiles, untiled host-side.
- AllGather split in 2 token-chunks pipelined into the vocab phase.
"""
import math
import os

import numpy as np
import ml_dtypes

import concourse.bass as bass
import concourse.tile as tile
from concourse import bacc, mybir
from concourse import bass_utils
from concourse.masks import make_identity

F32 = mybir.dt.float32
F16 = mybir.dt.float16
I32 = mybir.dt.int32
AF = mybir.ActivationFunctionType

D = 512
T = 256
H = 8
DK = 64
L = 6
DFF = 2048
V = 50257
B = 8
NCORES = 8
P = 128

TT = 2              # token tiles (T / P)
KB = D // P         # 4 contraction chunks over D
FB = DFF // P       # 16 chunks over DFF
VS = 6284           # per-core vocab shard (8 * 6284 = 50272 >= 50257)
VC = 50             # padded vocab 128-chunks per core (6400)
TS = 4              # 512-token slots in vocab matmul
NEGMASK = -60000.0  # fp16-safe -inf surrogate

_CACHE: dict = {}


def _steer_act_tables():
    """Make the ACT-table-load pass resolve every activation to the one
    set that contains all funcs this kernel uses (exp, ln, relu, identity,
    copy). Without this, Ln and Exp resolve to two different sets and the
    program ping-pongs table loads (~1.3us each) every layernorm."""
    from concourse import hw_specs
    if getattr(hw_specs.get_activation_tables, "_steered", False):
        return
    orig = hw_specs.get_activation_tables
    KEEP = "natural_log_exp_and_others"

    @__import__("functools").cache
    def patched(arch):
        tabs = dict(orig(arch))
        return {name: (funcs if name == KEEP else set())
                for name, funcs in tabs.items()}

    patched._steered = True
    hw_specs.get_activation_tables = patched
    bacc.get_activation_tables = patched


def _build_program():
    _steer_act_tables()
    nc = bacc.Bacc("TRN2", target_bir_lowering=False, debug=False,
                   num_devices=NCORES)

    # ---- I/O ----
    idx_h = nc.dram_tensor("idx", [T, 1], I32, kind="ExternalInput")
    emb_h = nc.dram_tensor("emb", [V, D], F16, kind="ExternalInput")
    posenc_h = nc.dram_tensor("posenc", [T, D], F32, kind="ExternalInput")
    mask_h = nc.dram_tensor("maskadd", [TT, P, T], F16, kind="ExternalInput")
    wqkv_h = nc.dram_tensor("wqkv", [L, 3, P, KB, D], F16, kind="ExternalInput")
    wo_h = nc.dram_tensor("wo", [L, P, KB, D], F16, kind="ExternalInput")
    w1_h = nc.dram_tensor("w1", [L, P, KB, DFF], F16, kind="ExternalInput")
    w2_h = nc.dram_tensor("w2", [L, P, FB, D], F16, kind="ExternalInput")
    # packed per-layer small consts: cq[4], ck[4], b1'[16]  (per partition)
    smallw_h = nc.dram_tensor("smallw", [L, P, 2 * KB + FB], F32,
                              kind="ExternalInput")
    # per-layer single-row consts: cv[512], b2[512]
    rows_h = nc.dram_tensor("rows", [L, 2, D], F16, kind="ExternalInput")
    wout_h = nc.dram_tensor("wout", [VC, P, KB * P], F16, kind="ExternalInput")
    bout_h = nc.dram_tensor("bout", [P, VC], F32, kind="ExternalInput")
    logits_h = nc.dram_tensor("logits", [VC, TS, P, 512], F16,
                              kind="ExternalOutput")

    scale = 1.0 / math.sqrt(D)

    with tile.TileContext(nc) as tc:
        from contextlib import ExitStack
        with ExitStack() as ctx:
            consts = ctx.enter_context(tc.tile_pool(name="consts", bufs=1))
            acts = ctx.enter_context(tc.tile_pool(name="acts", bufs=1))
            scr = ctx.enter_context(tc.tile_pool(name="scr", bufs=4))
            scr2 = ctx.enter_context(tc.tile_pool(name="scr2", bufs=4))
            dram = ctx.enter_context(tc.tile_pool(name="dram", bufs=1, space="DRAM"))

            # ---- constants ----
            identB = consts.tile([P, P], F16)
            make_identity(nc, identB)
            identF = consts.tile([P, P], F32)
            make_identity(nc, identF)
            ones1 = consts.tile([1, P], F16)
            nc.vector.memset(ones1, 1.0)
            eps_sb = consts.tile([P, 1], F32)
            nc.vector.memset(eps_sb, 1e-5)
            posenc_sb = consts.tile([P, TT, D], F32)
            nc.sync.dma_start(out=posenc_sb,
                              in_=posenc_h.ap().rearrange("(t p) d -> p t d", p=P))
            mask_sb = consts.tile([P, TT, T], F16)
            nc.sync.dma_start(out=mask_sb,
                              in_=mask_h.ap().rearrange("t p s -> p t s"))
            bout_sb = consts.tile([P, VC], F32)
            nc.sync.dma_start(out=bout_sb, in_=bout_h.ap())

            # ---- persistent activations ----
            xn = acts.tile([P, TT, D], F16)
            xnT = acts.tile([P, KB, T], F16)
            qt = acts.tile([P, KB, T], F16)
            kt = acts.tile([P, KB, T], F16)
            vv = acts.tile([P, TT, D], F16)
            ot = acts.tile([P, KB, T], F16)
            ht = acts.tile([P, FB, T], F16)

            # ---- PSUM pools (layer phase; closed before the vocab phase) ----
            lps = ExitStack()
            xp_pool = lps.enter_context(tc.tile_pool(name="xp", bufs=1, space="PSUM"))
            pA = lps.enter_context(tc.tile_pool(name="pA", bufs=3, space="PSUM"))
            pB = lps.enter_context(tc.tile_pool(name="pB", bufs=3, space="PSUM"))
            xp = xp_pool.tile([P, TT, D], F32)   # residual stream (2 banks)

            # ---- embedding + positional ----
            idx_sb = acts.tile([P, TT], I32)
            nc.sync.dma_start(out=idx_sb,
                              in_=idx_h.ap().rearrange("(t p) one -> p (t one)", p=P))
            for t in range(TT):
                emb_g = scr.tile([P, D], F16, name="emb_g")
                nc.gpsimd.indirect_dma_start(
                    out=emb_g[:], out_offset=None,
                    in_=emb_h.ap(),
                    in_offset=bass.IndirectOffsetOnAxis(ap=idx_sb[:, t:t + 1], axis=0),
                )
                xe = scr.tile([P, D], F32, name="xe")
                nc.vector.tensor_add(out=xe, in0=emb_g, in1=posenc_sb[:, t])
                # x[:, t] = xe  (via fp32 identity matmul; sets has_written)
                nc.tensor.matmul(xp[:, t], identF[:], xe[:], start=True, stop=True)

            def layernorm(t, src=None):
                """(x[:,t]-mean)*rstd -> xn[:,t] (fp16). rstd via ln+exp."""
                xs = xp if src is None else src
                stats = scr.tile([P, 6], F32, name="ln_stats")
                nc.vector.bn_stats(out=stats, in_=xs[:, t])
                mv = scr.tile([P, 2], F32, name="ln_mv")
                nc.vector.bn_aggr(out=mv, in_=stats)
                lv = scr.tile([P, 1], F32, name="ln_lv")
                nc.scalar.activation(out=lv, in_=mv[:, 1:2], func=AF.Ln,
                                     bias=eps_sb, scale=1.0)
                rstd = scr.tile([P, 1], F32, name="ln_rstd")
                nc.scalar.activation(out=rstd, in_=lv, func=AF.Exp,
                                     scale=-0.5)
                nc.vector.tensor_scalar(out=xn[:, t], in0=xs[:, t],
                                        scalar1=mv[:, 0:1], scalar2=rstd,
                                        op0=mybir.AluOpType.subtract,
                                        op1=mybir.AluOpType.mult)

            def transpose_tile(t):
                """xn[:, t] -> xnT[:, :, t*128:(t+1)*128]"""
                for kb in range(KB):
                    tp = pA.tile([P, P], F16, name="psA")
                    nc.tensor.transpose(out=tp[:],
                                        in_=xn[:, t, kb * P:(kb + 1) * P],
                                        identity=identB[:])
                    nc.any.tensor_copy(out=xnT[:, kb, t * P:(t + 1) * P], in_=tp[:])

            # ================= decoder layers =================
            LEFF = 0 if os.environ.get("KERNEL_BISECT") == "nolayers" else L
            with tc.tile_pool(name="wpool", bufs=2) as wp:
                for l in range(LEFF):
                    wqkv_t = wp.tile([P, 3, KB, D], F16, name="wqkv_t")
                    for m in range(3):
                        nc.sync.dma_start(out=wqkv_t[:, m], in_=wqkv_h.ap()[l, m])
                    wo_t = wp.tile([P, KB, D], F16, name="wo_t")
                    nc.sync.dma_start(out=wo_t, in_=wo_h.ap()[l])
                    w1_t = wp.tile([P, KB, DFF], F16, name="w1_t")
                    nc.sync.dma_start(out=w1_t, in_=w1_h.ap()[l])
                    w2_t = wp.tile([P, FB, D], F16, name="w2_t")
                    nc.sync.dma_start(out=w2_t, in_=w2_h.ap()[l])
                    sw = wp.tile([P, 2 * KB + FB], F32, name="sw")
                    nc.sync.dma_start(out=sw, in_=smallw_h.ap()[l])
                    rows_sb = wp.tile([1, 2, D], F16, name="rows_sb")
                    nc.sync.dma_start(out=rows_sb, in_=rows_h.ap()[l])

                    # ---- LN1 + transpose ----
                    for t in range(TT):
                        layernorm(t)
                        transpose_tile(t)

                    # ---- Q^T, K^T with folded-LN bias on the drain ----
                    for m, dst in ((0, qt), (1, kt)):
                        for pair in range(KB):
                            ps = pA.tile([P, T], F32, name="psA")
                            for kb in range(KB):
                                nc.tensor.matmul(
                                    ps[:],
                                    wqkv_t[:, m, kb, pair * P:(pair + 1) * P],
                                    xnT[:, kb],
                                    start=(kb == 0), stop=(kb == KB - 1))
                            bias_ap = sw[:, m * KB + pair:m * KB + pair + 1]
                            if pair % 2 == 0:
                                nc.scalar.activation(out=dst[:, pair], in_=ps[:],
                                                     func=AF.Identity,
                                                     bias=bias_ap, scale=1.0)
                            else:
                                nc.vector.tensor_scalar_add(out=dst[:, pair],
                                                            in0=ps[:],
                                                            scalar1=bias_ap)
                    # ---- V natural (+cv via K=1 ones-matmul) ----
                    for t in range(TT):
                        ps = pB.tile([P, D], F32, name="psB")
                        for kb in range(KB):
                            nc.tensor.matmul(ps[:], xnT[:, kb, t * P:(t + 1) * P],
                                             wqkv_t[:, 2, kb],
                                             start=(kb == 0), stop=False)
                        nc.tensor.matmul(ps[:], ones1[:],
                                         rows_sb[:, 0], start=False, stop=True)
                        nc.any.tensor_copy(out=vv[:, t], in_=ps[:])

                    # ---- attention ----
                    for pair in range(KB):
                        at_pair = []
                        for sub in range(2):
                            off = sub * DK
                            s_ps = pB.tile([P, TT, T], F32, name="psB")
                            # tq0: mask preload then scores over tk0 only
                            nc.tensor.matmul(s_ps[:, 0, 0:P], identB[:],
                                             mask_sb[:, 0, 0:P],
                                             start=True, stop=False)
                            nc.tensor.matmul(
                                s_ps[:, 0, 0:P],
                                qt[off:off + DK, pair, 0:P],
                                kt[off:off + DK, pair, 0:P],
                                start=False, stop=True)
                            # tq1: mask preload then scores over full tk
                            nc.tensor.matmul(s_ps[:, 1], identB[:],
                                             mask_sb[:, 1],
                                             start=True, stop=False)
                            nc.tensor.matmul(
                                s_ps[:, 1],
                                qt[off:off + DK, pair, P:T],
                                kt[off:off + DK, pair],
                                start=False, stop=True)

                            at = scr2.tile([P, TT, T], F16, name="at")
                            for tq in range(TT):
                                ncols = P if tq == 0 else T
                                pexp = scr.tile([P, T], F32, name="pexp")
                                den = scr.tile([P, 1], F32, name="den")
                                nc.scalar.activation(
                                    out=pexp[:, :ncols], in_=s_ps[:, tq, :ncols],
                                    func=AF.Exp, scale=scale, accum_out=den)
                                rden = scr.tile([P, 1], F32, name="rden")
                                nc.vector.reciprocal(out=rden, in_=den)
                                a_bf = scr.tile([P, T], F16, name="a_bf")
                                nc.vector.tensor_scalar_mul(out=a_bf[:, :ncols],
                                                            in0=pexp[:, :ncols],
                                                            scalar1=rden)
                                for tk in range(tq + 1):
                                    tp = pA.tile([P, P], F16, name="psA")
                                    nc.tensor.transpose(
                                        out=tp[:],
                                        in_=a_bf[:, tk * P:(tk + 1) * P],
                                        identity=identB[:])
                                    nc.any.tensor_copy(
                                        out=at[:, tk, tq * P:(tq + 1) * P],
                                        in_=tp[:])
                            at_pair.append(at)
                        # O^T for both heads (col-packed: out partitions 0-63 / 64-127)
                        o_ps = pA.tile([P, T], F32, name="psA")
                        for sub in range(2):
                            off = sub * DK
                            h = pair * 2 + sub
                            at = at_pair[sub]
                            nc.tensor.matmul(
                                o_ps[off:off + DK, :],
                                vv[:, 0, h * DK:(h + 1) * DK],
                                at[:, 0],
                                start=True, stop=False)
                            nc.tensor.matmul(
                                o_ps[off:off + DK, P:T],
                                vv[:, 1, h * DK:(h + 1) * DK],
                                at[:, 1, P:T],
                                start=False, stop=True)
                        nc.any.tensor_copy(out=ot[:, pair], in_=o_ps[:])

                    # ---- x += O @ Wo (accumulate straight into residual PSUM) ----
                    for tq in range(TT):
                        for kb in range(KB):
                            nc.tensor.matmul(xp[:, tq],
                                             ot[:, kb, tq * P:(tq + 1) * P],
                                             wo_t[:, kb],
                                             start=False, stop=(kb == KB - 1),
                                             skip_group_check=True)

                    # ---- LN2 + FFN ----
                    for t in range(TT):
                        layernorm(t)
                        transpose_tile(t)
                    for fc in range(FB):
                        ps = pA.tile([P, T], F32, name="psA")
                        for kb in range(KB):
                            nc.tensor.matmul(ps[:],
                                             w1_t[:, kb, fc * P:(fc + 1) * P],
                                             xnT[:, kb],
                                             start=(kb == 0), stop=(kb == KB - 1))
                        bias_ap = sw[:, 2 * KB + fc:2 * KB + fc + 1]
                        if fc % 2 == 0:
                            nc.scalar.activation(out=ht[:, fc], in_=ps[:],
                                                 func=AF.Relu, bias=bias_ap,
                                                 scale=1.0)
                        else:
                            nc.vector.tensor_scalar(out=ht[:, fc], in0=ps[:],
                                                    scalar1=bias_ap, scalar2=0.0,
                                                    op0=mybir.AluOpType.add,
                                                    op1=mybir.AluOpType.max)
                    for tq in range(TT):
                        for fc in range(FB):
                            nc.tensor.matmul(xp[:, tq],
                                             ht[:, fc, tq * P:(tq + 1) * P],
                                             w2_t[:, fc],
                                             start=False, stop=False,
                                             skip_group_check=True)
                        nc.tensor.matmul(xp[:, tq], ones1[:],
                                         rows_sb[:, 1], start=False, stop=True,
                                         skip_group_check=True)

            # ================= final LN + chunked all-gather =================
            # Copy the residual out of PSUM first: the vocab pool reuses the
            # xp banks, and a PE write there while final-LN still reads xp
            # (PE-W + DVE-R same bank) is fatal on HW. Reading from SBUF
            # chains every PSUM reader ahead of the vocab matmuls.
            x_sb = acts.tile([P, TT, D], F32)
            for t in range(TT):
                nc.vector.tensor_copy(out=x_sb[:, t], in_=xp[:, t])
            ag_in = [dram.tile([D, P], F16, name=f"ag_in{c}") for c in range(2)]
            ag_out = [dram.tile([NCORES * D, P], F16, addr_space="Shared",
                                name=f"ag_out{c}") for c in range(2)]
            for t in range(TT):
                layernorm(t, src=x_sb)
                transpose_tile(t)
                for kb in range(KB):
                    nc.sync.dma_start(out=ag_in[t][kb * P:(kb + 1) * P, :],
                                      in_=xnT[:, kb, t * P:(t + 1) * P])
                nc.gpsimd.collective_compute(
                    "AllGather", mybir.AluOpType.bypass,
                    replica_groups=[list(range(NCORES))],
                    ins=[ag_in[t][:]], outs=[ag_out[t][:]])
            lps.close()   # free layer-phase PSUM for the vocab pool

            # ================= vocab projection (transposed) =================
            with tc.tile_pool(name="vw", bufs=6) as vw, \
                 tc.tile_pool(name="vo", bufs=6) as vo, \
                 tc.tile_pool(name="vg", bufs=1) as vg, \
                 tc.tile_pool(name="vps", bufs=2, space="PSUM") as vps:
                xg = vg.tile([P, KB, B * T], F16)
                for c in range(2):
                    src = ag_out[c][:].rearrange(
                        "(b kb p) t -> p kb b t", b=B, kb=KB, p=P)
                    for kb in range(KB):
                        nc.sync.dma_start(
                            out=xg[:, kb, c * 1024:(c + 1) * 1024],
                            in_=src[:, kb])
                VCEFF = 1 if os.environ.get("KERNEL_BISECT") == "novocab" else VC
                for vc in range(VCEFF):
                    wsb = vw.tile([P, KB, P], F16, name="wsb")
                    nc.sync.dma_start(
                        out=wsb,
                        in_=wout_h.ap()[vc].rearrange("p (kb n) -> p kb n", kb=KB))
                    ps = vps.tile([P, TS, 512], F32, name="vps")
                    for kb in range(KB):
                        for ts in range(TS):
                            nc.tensor.matmul(
                                ps[:, ts], wsb[:, kb],
                                xg[:, kb, ts * 512:(ts + 1) * 512],
                                start=(kb == 0), stop=(kb == KB - 1))
                    for ts in range(TS):
                        lg = vo.tile([P, 512], F16, name="lg")
                        nc.scalar.activation(out=lg, in_=ps[:, ts],
                                             func=AF.Identity,
                                             bias=bout_sb[:, vc:vc + 1], scale=1.0)
                        nc.sync.dma_start(out=logits_h.ap()[vc, ts], in_=lg)

    nc.compile()
    return nc


def _prep_inputs(inputs):
    """Host-side shard/cast/layout with exact LN folding."""
    f32 = np.float32
    f16 = np.float16

    idx = np.asarray(inputs["idx"])
    emb = np.asarray(inputs["emb"], f32)

    pos = np.arange(T, dtype=np.float64)[:, None]
    div = np.exp(np.arange(0, D, 2, dtype=np.float64) * (-math.log(10000.0) / D))
    pe = np.zeros((T, D), f32)
    pe[:, 0::2] = np.sin(pos * div).astype(f32)
    pe[:, 1::2] = np.cos(pos * div).astype(f32)

    # mask tiles: [TT, P, T]; tile tq row p masks cols > tq*128+p
    maskadd = np.zeros((TT, P, T), f32)
    for tq in range(TT):
        for p in range(P):
            maskadd[tq, p, tq * P + p + 1:] = NEGMASK

    wq = np.asarray(inputs["Wq"], f32)  # [L, H, D, DK]
    wk = np.asarray(inputs["Wk"], f32)
    wv = np.asarray(inputs["Wv"], f32)
    ln1g = np.asarray(inputs["ln1_g"], f32)  # [L, D]
    ln1b = np.asarray(inputs["ln1_b"], f32)
    ln2g = np.asarray(inputs["ln2_g"], f32)
    ln2b = np.asarray(inputs["ln2_b"], f32)
    w1 = np.asarray(inputs["W1"], f32)       # [L, D, DFF]
    b1 = np.asarray(inputs["b1"], f32)       # [L, DFF]
    w2 = np.asarray(inputs["W2"], f32)
    b2 = np.asarray(inputs["b2"], f32)
    lnfg = np.asarray(inputs["lnf_g"], f32)
    lnfb = np.asarray(inputs["lnf_b"], f32)

    # concat heads: [L, D, D], then fold ln1 gain into rows
    wq_c = wq.transpose(0, 2, 1, 3).reshape(L, D, D)
    wk_c = wk.transpose(0, 2, 1, 3).reshape(L, D, D)
    wv_c = wv.transpose(0, 2, 1, 3).reshape(L, D, D)
    g1 = ln1g[:, :, None]
    wqkv = np.stack([wq_c * g1, wk_c * g1, wv_c * g1], axis=1)  # [L,3,D,D]
    cq = np.einsum('ld,ldo->lo', ln1b, wq_c)   # [L, D]
    ck = np.einsum('ld,ldo->lo', ln1b, wk_c)
    cv = np.einsum('ld,ldo->lo', ln1b, wv_c)
    w1_f = w1 * ln2g[:, :, None]
    b1_f = b1 + np.einsum('ld,ldf->lf', ln2b, w1)

    wqkv_t = np.ascontiguousarray(
        wqkv.reshape(L, 3, KB, P, D).transpose(0, 1, 3, 2, 4)).astype(f16)
    wo_t = np.ascontiguousarray(
        np.asarray(inputs["Wo"], f32).reshape(L, KB, P, D)
        .transpose(0, 2, 1, 3)).astype(f16)
    w1_t = np.ascontiguousarray(
        w1_f.reshape(L, KB, P, DFF).transpose(0, 2, 1, 3)).astype(f16)
    w2_t = np.ascontiguousarray(
        np.asarray(inputs["W2"], f32).reshape(L, FB, P, D)
        .transpose(0, 2, 1, 3)).astype(f16)

    # smallw: [L, P, 2*KB+FB] = cq tiles, ck tiles, b1' tiles
    smallw = np.zeros((L, P, 2 * KB + FB), f32)
    smallw[:, :, 0:KB] = cq.reshape(L, KB, P).transpose(0, 2, 1)
    smallw[:, :, KB:2 * KB] = ck.reshape(L, KB, P).transpose(0, 2, 1)
    smallw[:, :, 2 * KB:] = b1_f.reshape(L, FB, P).transpose(0, 2, 1)

    rows = np.stack([cv, b2], axis=1).astype(f16)  # [L, 2, D]

    # vocab: fold final LN gain/bias, pad to 8*6400
    wout = np.asarray(inputs["Wout"], f32)
    bout = np.asarray(inputs["bout"], f32)
    wout_f = lnfg[:, None] * wout
    bout_f = bout + lnfb @ wout
    VPAD = VC * P * NCORES
    wout_pad = np.zeros((D, VPAD), f32)
    bout_pad = np.zeros((VPAD,), f32)
    for c in range(NCORES):
        lo, hi = c * VS, min((c + 1) * VS, V)
        if lo < V:
            wout_pad[:, c * VC * P:c * VC * P + (hi - lo)] = wout_f[:, lo:hi]
            bout_pad[c * VC * P:c * VC * P + (hi - lo)] = bout_f[lo:hi]

    common = dict(
        emb=emb.astype(f16), posenc=pe, maskadd=maskadd.astype(f16),
        wqkv=wqkv_t, wo=wo_t, w1=w1_t, w2=w2_t,
        smallw=smallw, rows=rows,
    )
    in_maps = []
    for c in range(NCORES):
        m = dict(common)
        m["idx"] = np.ascontiguousarray(idx[c].astype(np.int32).reshape(T, 1))
        ws = wout_pad[:, c * VC * P:(c + 1) * VC * P]  # [D, 6400]
        # wout_h[vc, p, kb*128+n] = ws[kb*128+p, vc*128+n]
        w4 = ws.reshape(KB, P, VC, P).transpose(2, 1, 0, 3).reshape(VC, P, KB * P)
        m["wout"] = np.ascontiguousarray(w4).astype(f16)
        bs = bout_pad[c * VC * P:(c + 1) * VC * P]
        m["bout"] = np.ascontiguousarray(bs.reshape(VC, P).T)
        in_maps.append(m)
    return in_maps


def _unshard(results):
    full = np.zeros((B, T, NCORES * VC * P), np.float32)
    for c in range(NCORES):
        arr = np.asarray(results[c]["logits"], np.float32)  # [VC, TS, P, 512]
        # col within ts-block: (b%4)*128 + t_local; ts = 2*chunk + b//4
        a = arr.reshape(VC, 2, 2, P, 4, P)      # (vc, c2, h2, p, b4, tl)
        a = a.transpose(2, 4, 1, 5, 0, 3)       # (h2, b4, c2, tl, vc, p)
        full[:, :, c * VC * P:(c + 1) * VC * P] = a.reshape(B, T, VC * P)
    out = np.zeros((B, T, V), np.float32)
    for c in range(NCORES):
        lo, hi = c * VS, min((c + 1) * VS, V)
        if lo < V:
            out[:, :, lo:hi] = full[:, :, c * VC * P:c * VC * P + (hi - lo)]
    return out


def kernel(**inputs):
    if "nc" not in _CACHE:
        _CACHE["nc"] = _build_program()
    nc = _CACHE["nc"]
    in_maps = _prep_inputs(inputs)

    if os.environ.get("KERNEL_USE_SIM"):
        from concourse.bass_interp import MultiCoreSim
        sim = MultiCoreSim(nc, num_cores=NCORES,
                           num_workers=int(os.environ.get("KERNEL_SIM_WORKERS", "8")))
        for c in range(NCORES):
            for name, val in in_maps[c].items():
                sim.cores[c].tensor(name)[:] = val
        sim.simulate()
        results = [
            {"logits": np.array(sim.cores[c].tensor("logits"))}
            for c in range(NCORES)
        ]
        return _unshard(results)

    res = bass_utils.run_bass_kernel_spmd(
        nc, in_maps, core_ids=list(range(NCORES)))
    return _unshard(res.results)


# revision 23
# speedup vs baseline: 1.2865x; 1.0448x over previous
"""GPT decoder (B=8,T=256,D=512,H=8,L=6,DFF=2048,V=50257) on 8 TRN2 NeuronCores.

Strategy (v3):
- Data-parallel over batch (core c owns batch c); vocab projection is
  tensor-parallel over vocab (per-core shard 6284, padded to 6400).
- LN gains/biases folded into weights host-side (exact algebra); on-chip LN
  only computes (x-mean)*rstd, with rstd = exp(-0.5*ln(var+eps)) so the
  whole kernel uses a single ACT table set (natural_log_exp_and_others).
- Residual stream lives in PSUM (xp0/xp1, one bank per token tile):
  Wo/W2/bias adds are matmul accumulations; LN reads PSUM directly.
- Tiles are split per token-tile / head-pair / dff-chunk so Tile's
  tile-granular dependency tracking doesn't serialize independent work
  (LN of tile 0 overlaps the previous FFN's tile-1 matmuls, etc).
- Causal skip; causal mask preloaded into PSUM via identity-matmul.
- fp16 activations/weights; fp16 logits (halved output DMA).
- Vocab projection computed transposed (logits^T) so bout is a
  per-partition bias on the drains (split ACT/DVE); Wout tile stationary
  across 4 token-slot matmuls; contiguous fp16 output tiles.
- AllGather split in 2 token-chunks; vocab runs chunk-0 token slots first
  so chunk 1 gathers behind compute.
"""
import math
import os

import numpy as np

import concourse.bass as bass
import concourse.tile as tile
from concourse import bacc, mybir
from concourse import bass_utils
from concourse.masks import make_identity

F32 = mybir.dt.float32
F16 = mybir.dt.float16
I32 = mybir.dt.int32
AF = mybir.ActivationFunctionType

D = 512
T = 256
H = 8
DK = 64
L = 6
DFF = 2048
V = 50257
B = 8
NCORES = 8
P = 128

TT = 2              # token tiles (T / P)
KB = D // P         # 4 contraction chunks over D
FB = DFF // P       # 16 chunks over DFF
VS = 6284           # per-core vocab shard (8 * 6284 = 50272 >= 50257)
VC = 50             # padded vocab 128-chunks per core (6400)
TS = 4              # 512-token slots in vocab matmul
VEARLY = 6          # vocab chunks that run ts{0,1} first (hide AG chunk 1)
NEGMASK = -60000.0  # fp16-safe -inf surrogate

_CACHE: dict = {}


def _steer_act_tables():
    """Make the ACT-table-load pass resolve every activation to the one
    set that contains all funcs this kernel uses (exp, ln, relu, identity,
    copy). Without this, Ln and Exp resolve to two different sets and the
    program ping-pongs table loads (~1.3us each) every layernorm."""
    from concourse import hw_specs
    if getattr(hw_specs.get_activation_tables, "_steered", False):
        return
    orig = hw_specs.get_activation_tables
    KEEP = "natural_log_exp_and_others"

    @__import__("functools").cache
    def patched(arch):
        tabs = dict(orig(arch))
        return {name: (funcs if name == KEEP else set())
                for name, funcs in tabs.items()}

    patched._steered = True
    hw_specs.get_activation_tables = patched
    bacc.get_activation_tables = patched


def _build_program():
    _steer_act_tables()
    nc = bacc.Bacc("TRN2", target_bir_lowering=False, debug=False,
                   num_devices=NCORES)

    # ---- I/O ----
    idx_h = nc.dram_tensor("idx", [T, 1], I32, kind="ExternalInput")
    emb_h = nc.dram_tensor("emb", [V, D], F16, kind="ExternalInput")
    posenc_h = nc.dram_tensor("posenc", [T, D], F32, kind="ExternalInput")
    mask_h = nc.dram_tensor("maskadd", [TT, P, T], F16, kind="ExternalInput")
    wqkv_h = nc.dram_tensor("wqkv", [L, 3, P, KB, D], F16, kind="ExternalInput")
    wo_h = nc.dram_tensor("wo", [L, P, KB, D], F16, kind="ExternalInput")
    w1_h = nc.dram_tensor("w1", [L, P, KB, DFF], F16, kind="ExternalInput")
    w2_h = nc.dram_tensor("w2", [L, P, FB, D], F16, kind="ExternalInput")
    # packed per-layer small consts: cq[4], ck[4], b1'[16]  (per partition)
    smallw_h = nc.dram_tensor("smallw", [L, P, 2 * KB + FB], F32,
                              kind="ExternalInput")
    # per-layer single-row consts: cv[512], b2[512]
    rows_h = nc.dram_tensor("rows", [L, 2, D], F16, kind="ExternalInput")
    wout_h = nc.dram_tensor("wout", [VC, P, KB * P], F16, kind="ExternalInput")
    bout_h = nc.dram_tensor("bout", [P, VC], F32, kind="ExternalInput")
    logits_h = nc.dram_tensor("logits", [VC, TS, P, 512], F16,
                              kind="ExternalOutput")

    scale = 1.0 / math.sqrt(D)

    with tile.TileContext(nc) as tc:
        from contextlib import ExitStack
        with ExitStack() as ctx:
            consts = ctx.enter_context(tc.tile_pool(name="consts", bufs=1))
            acts = ctx.enter_context(tc.tile_pool(name="acts", bufs=1))
            scr = ctx.enter_context(tc.tile_pool(name="scr", bufs=4))
            scr2 = ctx.enter_context(tc.tile_pool(name="scr2", bufs=4))
            dram = ctx.enter_context(tc.tile_pool(name="dram", bufs=1, space="DRAM"))

            # ---- constants ----
            identB = consts.tile([P, P], F16)
            make_identity(nc, identB)
            identF = consts.tile([P, P], F32)
            make_identity(nc, identF)
            ones1 = consts.tile([1, P], F16)
            nc.vector.memset(ones1, 1.0)
            eps_sb = consts.tile([P, 1], F32)
            nc.vector.memset(eps_sb, 1e-5)
            posenc_sb = consts.tile([P, TT, D], F32)
            nc.sync.dma_start(out=posenc_sb,
                              in_=posenc_h.ap().rearrange("(t p) d -> p t d", p=P))
            mask_sb = consts.tile([P, TT, T], F16)
            nc.sync.dma_start(out=mask_sb,
                              in_=mask_h.ap().rearrange("t p s -> p t s"))
            bout_sb = consts.tile([P, VC], F32)
            nc.sync.dma_start(out=bout_sb, in_=bout_h.ap())

            # ---- persistent activations (split to avoid false deps) ----
            xns = [acts.tile([P, D], F16, name=f"xn{t}") for t in range(TT)]
            xnT = [acts.tile([P, T], F16, name=f"xnT{k}") for k in range(KB)]
            qtp = [acts.tile([P, T], F16, name=f"qtp{k}") for k in range(KB)]
            ktp = [acts.tile([P, T], F16, name=f"ktp{k}") for k in range(KB)]
            vvs = [acts.tile([P, D], F16, name=f"vv{t}") for t in range(TT)]
            otp = [acts.tile([P, T], F16, name=f"otp{k}") for k in range(KB)]
            htf = [acts.tile([P, T], F16, name=f"htf{f}") for f in range(FB)]
            xsb = [acts.tile([P, D], F32, name=f"xsb{t}") for t in range(TT)]

            # ---- PSUM pools (layer phase; closed before the vocab phase) ----
            lps = ExitStack()
            xp_pool = lps.enter_context(tc.tile_pool(name="xp", bufs=1, space="PSUM"))
            pA = lps.enter_context(tc.tile_pool(name="pA", bufs=3, space="PSUM"))
            pB = lps.enter_context(tc.tile_pool(name="pB", bufs=3, space="PSUM"))
            xps = [xp_pool.tile([P, D], F32, name=f"xp{t}") for t in range(TT)]

            # ---- embedding + positional ----
            idx_sb = acts.tile([P, TT], I32)
            nc.sync.dma_start(out=idx_sb,
                              in_=idx_h.ap().rearrange("(t p) one -> p (t one)", p=P))
            for t in range(TT):
                emb_g = scr.tile([P, D], F16, name="emb_g")
                nc.gpsimd.indirect_dma_start(
                    out=emb_g[:], out_offset=None,
                    in_=emb_h.ap(),
                    in_offset=bass.IndirectOffsetOnAxis(ap=idx_sb[:, t:t + 1], axis=0),
                )
                xe = scr.tile([P, D], F32, name="xe")
                nc.vector.tensor_add(out=xe, in0=emb_g, in1=posenc_sb[:, t])
                # x[t] = xe  (fp32 identity matmul; sets has_written)
                nc.tensor.matmul(xps[t][:], identF[:], xe[:], start=True, stop=True)

            def layernorm(t, src=None):
                """(x[t]-mean)*rstd -> xns[t] (fp16). rstd via ln+exp.
                t==0 normalizes on DVE; t==1 on ACT (parallel engines)."""
                xs = xps[t] if src is None else src[t]
                stats = scr.tile([P, 6], F32, name="ln_stats")
                nc.vector.bn_stats(out=stats, in_=xs[:])
                mv = scr.tile([P, 2], F32, name="ln_mv")
                nc.vector.bn_aggr(out=mv, in_=stats)
                lv = scr.tile([P, 1], F32, name="ln_lv")
                nc.scalar.activation(out=lv, in_=mv[:, 1:2], func=AF.Ln,
                                     bias=eps_sb, scale=1.0)
                rstd = scr.tile([P, 1], F32, name="ln_rstd")
                nc.scalar.activation(out=rstd, in_=lv, func=AF.Exp,
                                     scale=-0.5)
                if t == 0:
                    nc.vector.tensor_scalar(out=xns[t][:], in0=xs[:],
                                            scalar1=mv[:, 0:1], scalar2=rstd,
                                            op0=mybir.AluOpType.subtract,
                                            op1=mybir.AluOpType.mult)
                else:
                    nb = scr.tile([P, 1], F32, name="ln_nb")
                    nc.vector.tensor_scalar(out=nb, in0=mv[:, 0:1],
                                            scalar1=rstd, scalar2=-1.0,
                                            op0=mybir.AluOpType.mult,
                                            op1=mybir.AluOpType.mult)
                    nc.scalar.activation(out=xns[t][:], in_=xs[:],
                                         func=AF.Identity, bias=nb, scale=rstd)

            def transpose_tile(t):
                """xns[t] -> xnT[kb][:, t*128:(t+1)*128]"""
                for kb in range(KB):
                    tp = pA.tile([P, P], F16, name="psA")
                    nc.tensor.transpose(out=tp[:],
                                        in_=xns[t][:, kb * P:(kb + 1) * P],
                                        identity=identB[:])
                    if kb % 2 == 0:
                        nc.vector.tensor_copy(out=xnT[kb][:, t * P:(t + 1) * P],
                                              in_=tp[:])
                    else:
                        nc.scalar.copy(out=xnT[kb][:, t * P:(t + 1) * P],
                                       in_=tp[:])

            # ================= decoder layers =================
            LEFF = 0 if os.environ.get("KERNEL_BISECT") == "nolayers" else L
            with tc.tile_pool(name="wpool", bufs=2) as wp:
                for l in range(LEFF):
                    wqkv_t = wp.tile([P, 3, KB, D], F16, name="wqkv_t")
                    for m in range(3):
                        nc.sync.dma_start(out=wqkv_t[:, m], in_=wqkv_h.ap()[l, m])
                    wo_t = wp.tile([P, KB, D], F16, name="wo_t")
                    nc.sync.dma_start(out=wo_t, in_=wo_h.ap()[l])
                    w1_t = wp.tile([P, KB, DFF], F16, name="w1_t")
                    nc.sync.dma_start(out=w1_t, in_=w1_h.ap()[l])
                    w2_t = wp.tile([P, FB, D], F16, name="w2_t")
                    nc.sync.dma_start(out=w2_t, in_=w2_h.ap()[l])
                    sw = wp.tile([P, 2 * KB + FB], F32, name="sw")
                    nc.sync.dma_start(out=sw, in_=smallw_h.ap()[l])
                    rows_sb = wp.tile([1, 2, D], F16, name="rows_sb")
                    nc.sync.dma_start(out=rows_sb, in_=rows_h.ap()[l])

                    # ---- LN1; V(t0) matmuls bridge the t1 LN chain ----
                    layernorm(0)
                    transpose_tile(0)
                    layernorm(1)
                    vps0 = pB.tile([P, D], F32, name="psB")
                    for kb in range(KB):
                        nc.tensor.matmul(vps0[:], xnT[kb][:, 0:P],
                                         wqkv_t[:, 2, kb],
                                         start=(kb == 0), stop=False)
                    nc.tensor.matmul(vps0[:], ones1[:], rows_sb[:, 0],
                                     start=False, stop=True)
                    nc.vector.tensor_copy(out=vvs[0][:], in_=vps0[:])
                    transpose_tile(1)

                    # ---- Q^T, K^T with folded-LN bias on the drain ----
                    for m, dst in ((0, qtp), (1, ktp)):
                        for pair in range(KB):
                            ps = pA.tile([P, T], F32, name="psA")
                            for kb in range(KB):
                                nc.tensor.matmul(
                                    ps[:],
                                    wqkv_t[:, m, kb, pair * P:(pair + 1) * P],
                                    xnT[kb][:],
                                    start=(kb == 0), stop=(kb == KB - 1))
                            bias_ap = sw[:, m * KB + pair:m * KB + pair + 1]
                            if pair % 2 == 0:
                                nc.scalar.activation(out=dst[pair][:], in_=ps[:],
                                                     func=AF.Identity,
                                                     bias=bias_ap, scale=1.0)
                            else:
                                nc.vector.tensor_scalar_add(out=dst[pair][:],
                                                            in0=ps[:],
                                                            scalar1=bias_ap)
                    # ---- V(t1) ----
                    vps1 = pB.tile([P, D], F32, name="psB")
                    for kb in range(KB):
                        nc.tensor.matmul(vps1[:], xnT[kb][:, P:T],
                                         wqkv_t[:, 2, kb],
                                         start=(kb == 0), stop=False)
                    nc.tensor.matmul(vps1[:], ones1[:], rows_sb[:, 0],
                                     start=False, stop=True)
                    nc.scalar.copy(out=vvs[1][:], in_=vps1[:])

                    # ---- attention ----
                    for pair in range(KB):
                        at_pair = []
                        for sub in range(2):
                            off = sub * DK
                            s_ps = pB.tile([P, TT, T], F32, name="psB")
                            nc.tensor.matmul(s_ps[:, 0, 0:P], identB[:],
                                             mask_sb[:, 0, 0:P],
                                             start=True, stop=False)
                            nc.tensor.matmul(
                                s_ps[:, 0, 0:P],
                                qtp[pair][off:off + DK, 0:P],
                                ktp[pair][off:off + DK, 0:P],
                                start=False, stop=True)
                            nc.tensor.matmul(s_ps[:, 1], identB[:],
                                             mask_sb[:, 1],
                                             start=True, stop=False)
                            nc.tensor.matmul(
                                s_ps[:, 1],
                                qtp[pair][off:off + DK, P:T],
                                ktp[pair][off:off + DK, :],
                                start=False, stop=True)

                            at = scr2.tile([P, TT, T], F16, name="at")
                            for tq in range(TT):
                                ncols = P if tq == 0 else T
                                pexp = scr.tile([P, T], F32, name="pexp")
                                den = scr.tile([P, 1], F32, name="den")
                                nc.scalar.activation(
                                    out=pexp[:, :ncols], in_=s_ps[:, tq, :ncols],
                                    func=AF.Exp, scale=scale, accum_out=den)
                                rden = scr.tile([P, 1], F32, name="rden")
                                nc.vector.reciprocal(out=rden, in_=den)
                                a_bf = scr.tile([P, T], F16, name="a_bf")
                                nc.vector.tensor_scalar_mul(out=a_bf[:, :ncols],
                                                            in0=pexp[:, :ncols],
                                                            scalar1=rden)
                                for tk in range(tq + 1):
                                    tp = pA.tile([P, P], F16, name="psA")
                                    nc.tensor.transpose(
                                        out=tp[:],
                                        in_=a_bf[:, tk * P:(tk + 1) * P],
                                        identity=identB[:])
                                    if (tq + tk) % 2 == 0:
                                        nc.vector.tensor_copy(
                                            out=at[:, tk, tq * P:(tq + 1) * P],
                                            in_=tp[:])
                                    else:
                                        nc.scalar.copy(
                                            out=at[:, tk, tq * P:(tq + 1) * P],
                                            in_=tp[:])
                            at_pair.append(at)
                        # O^T both heads (col-packed: out partitions 0-63/64-127)
                        o_ps = pA.tile([P, T], F32, name="psA")
                        for sub in range(2):
                            off = sub * DK
                            h = pair * 2 + sub
                            at = at_pair[sub]
                            nc.tensor.matmul(
                                o_ps[off:off + DK, :],
                                vvs[0][:, h * DK:(h + 1) * DK],
                                at[:, 0],
                                start=True, stop=False)
                            nc.tensor.matmul(
                                o_ps[off:off + DK, P:T],
                                vvs[1][:, h * DK:(h + 1) * DK],
                                at[:, 1, P:T],
                                start=False, stop=True)
                        if pair % 2 == 0:
                            nc.vector.tensor_copy(out=otp[pair][:], in_=o_ps[:])
                        else:
                            nc.scalar.copy(out=otp[pair][:], in_=o_ps[:])

                    # ---- x += O @ Wo (accumulate into residual PSUM) ----
                    for tq in range(TT):
                        for kb in range(KB):
                            nc.tensor.matmul(xps[tq][:],
                                             otp[kb][:, tq * P:(tq + 1) * P],
                                             wo_t[:, kb],
                                             start=False, stop=(kb == KB - 1),
                                             skip_group_check=True)

                    # ---- LN2 + FFN (W1/W2 interleaved per dff chunk) ----
                    layernorm(0)
                    transpose_tile(0)
                    layernorm(1)
                    transpose_tile(1)
                    for fc in range(FB):
                        ps = pA.tile([P, T], F32, name="psA")
                        for kb in range(KB):
                            nc.tensor.matmul(ps[:],
                                             w1_t[:, kb, fc * P:(fc + 1) * P],
                                             xnT[kb][:],
                                             start=(kb == 0), stop=(kb == KB - 1))
                        bias_ap = sw[:, 2 * KB + fc:2 * KB + fc + 1]
                        if fc % 2 == 0:
                            nc.scalar.activation(out=htf[fc][:], in_=ps[:],
                                                 func=AF.Relu, bias=bias_ap,
                                                 scale=1.0)
                        else:
                            nc.vector.tensor_scalar(out=htf[fc][:], in0=ps[:],
                                                    scalar1=bias_ap, scalar2=0.0,
                                                    op0=mybir.AluOpType.add,
                                                    op1=mybir.AluOpType.max)
                        for tq in range(TT):
                            nc.tensor.matmul(xps[tq][:],
                                             htf[fc][:, tq * P:(tq + 1) * P],
                                             w2_t[:, fc],
                                             start=False, stop=False,
                                             skip_group_check=True)
                    for tq in range(TT):
                        nc.tensor.matmul(xps[tq][:], ones1[:],
                                         rows_sb[:, 1], start=False, stop=True,
                                         skip_group_check=True)

            # ================= final LN + chunked all-gather =================
            # Residual leaves PSUM first: the vocab pool reuses the xp banks,
            # and a PE write there while a final-LN read is in flight
            # (PE-W + DVE-R same bank) is fatal on HW. Reading from SBUF
            # chains every PSUM reader ahead of the vocab matmuls.
            ag_in = [dram.tile([D, P], F16, name=f"ag_in{c}") for c in range(2)]
            ag_out = [dram.tile([NCORES * D, P], F16, addr_space="Shared",
                                name=f"ag_out{c}") for c in range(2)]
            for t in range(TT):
                nc.vector.tensor_copy(out=xsb[t][:], in_=xps[t][:])
                layernorm(t, src=xsb)
                transpose_tile(t)
                for kb in range(KB):
                    nc.sync.dma_start(out=ag_in[t][kb * P:(kb + 1) * P, :],
                                      in_=xnT[kb][:, t * P:(t + 1) * P])
                nc.gpsimd.collective_compute(
                    "AllGather", mybir.AluOpType.bypass,
                    replica_groups=[list(range(NCORES))],
                    ins=[ag_in[t][:]], outs=[ag_out[t][:]])
            lps.close()   # free layer-phase PSUM for the vocab pool

            # ================= vocab projection (transposed) =================
            with tc.tile_pool(name="vw", bufs=6) as vw, \
                 tc.tile_pool(name="vo", bufs=8) as vo, \
                 tc.tile_pool(name="vg", bufs=1) as vg, \
                 tc.tile_pool(name="vps", bufs=2, space="PSUM") as vps:
                xgs = [vg.tile([P, KB, 2 * 512], F16, name=f"xg{c}")
                       for c in range(2)]
                for c in range(2):
                    src = ag_out[c][:].rearrange(
                        "(b kb p) t -> p kb b t", b=B, kb=KB, p=P)
                    for kb in range(KB):
                        nc.sync.dma_start(out=xgs[c][:, kb], in_=src[:, kb])

                def vocab_chunk(vc, ts_list):
                    wsb = vw.tile([P, KB, P], F16, name="wsb")
                    nc.sync.dma_start(
                        out=wsb,
                        in_=wout_h.ap()[vc].rearrange("p (kb n) -> p kb n", kb=KB))
                    ps = vps.tile([P, TS, 512], F32, name="vps")
                    for kb in range(KB):
                        for ts in ts_list:
                            nc.tensor.matmul(
                                ps[:, ts], wsb[:, kb],
                                xgs[ts // 2][:, kb, (ts % 2) * 512:
                                             (ts % 2 + 1) * 512],
                                start=(kb == 0), stop=(kb == KB - 1))
                    for ts in ts_list:
                        lg = vo.tile([P, 512], F16, name="lg")
                        if ts % 2 == 0:
                            nc.scalar.activation(out=lg, in_=ps[:, ts],
                                                 func=AF.Identity,
                                                 bias=bout_sb[:, vc:vc + 1],
                                                 scale=1.0)
                        else:
                            nc.vector.tensor_scalar_add(out=lg, in0=ps[:, ts],
                                                        scalar1=bout_sb[:, vc:vc + 1])
                        nc.sync.dma_start(out=logits_h.ap()[vc, ts], in_=lg)

                if os.environ.get("KERNEL_BISECT") == "novocab":
                    vocab_chunk(0, [0, 1, 2, 3])
                else:
                    for vc in range(VEARLY):
                        vocab_chunk(vc, [0, 1])
                    for vc in range(VEARLY, VC):
                        vocab_chunk(vc, [0, 1, 2, 3])
                    for vc in range(VEARLY):
                        vocab_chunk(vc, [2, 3])

    nc.compile()
    return nc


def _prep_inputs(inputs):
    """Host-side shard/cast/layout with exact LN folding."""
    f32 = np.float32
    f16 = np.float16

    idx = np.asarray(inputs["idx"])
    emb = np.asarray(inputs["emb"], f32)

    pos = np.arange(T, dtype=np.float64)[:, None]
    div = np.exp(np.arange(0, D, 2, dtype=np.float64) * (-math.log(10000.0) / D))
    pe = np.zeros((T, D), f32)
    pe[:, 0::2] = np.sin(pos * div).astype(f32)
    pe[:, 1::2] = np.cos(pos * div).astype(f32)

    # mask tiles: [TT, P, T]; tile tq row p masks cols > tq*128+p
    maskadd = np.zeros((TT, P, T), f32)
    for tq in range(TT):
        for p in range(P):
            maskadd[tq, p, tq * P + p + 1:] = NEGMASK

    wq = np.asarray(inputs["Wq"], f32)  # [L, H, D, DK]
    wk = np.asarray(inputs["Wk"], f32)
    wv = np.asarray(inputs["Wv"], f32)
    ln1g = np.asarray(inputs["ln1_g"], f32)  # [L, D]
    ln1b = np.asarray(inputs["ln1_b"], f32)
    ln2g = np.asarray(inputs["ln2_g"], f32)
    ln2b = np.asarray(inputs["ln2_b"], f32)
    w1 = np.asarray(inputs["W1"], f32)       # [L, D, DFF]
    b1 = np.asarray(inputs["b1"], f32)       # [L, DFF]
    b2 = np.asarray(inputs["b2"], f32)
    lnfg = np.asarray(inputs["lnf_g"], f32)
    lnfb = np.asarray(inputs["lnf_b"], f32)

    # concat heads: [L, D, D], then fold ln1 gain into rows
    wq_c = wq.transpose(0, 2, 1, 3).reshape(L, D, D)
    wk_c = wk.transpose(0, 2, 1, 3).reshape(L, D, D)
    wv_c = wv.transpose(0, 2, 1, 3).reshape(L, D, D)
    g1 = ln1g[:, :, None]
    wqkv = np.stack([wq_c * g1, wk_c * g1, wv_c * g1], axis=1)  # [L,3,D,D]
    cq = np.einsum('ld,ldo->lo', ln1b, wq_c)   # [L, D]
    ck = np.einsum('ld,ldo->lo', ln1b, wk_c)
    cv = np.einsum('ld,ldo->lo', ln1b, wv_c)
    w1_f = w1 * ln2g[:, :, None]
    b1_f = b1 + np.einsum('ld,ldf->lf', ln2b, w1)

    wqkv_t = np.ascontiguousarray(
        wqkv.reshape(L, 3, KB, P, D).transpose(0, 1, 3, 2, 4)).astype(f16)
    wo_t = np.ascontiguousarray(
        np.asarray(inputs["Wo"], f32).reshape(L, KB, P, D)
        .transpose(0, 2, 1, 3)).astype(f16)
    w1_t = np.ascontiguousarray(
        w1_f.reshape(L, KB, P, DFF).transpose(0, 2, 1, 3)).astype(f16)
    w2_t = np.ascontiguousarray(
        np.asarray(inputs["W2"], f32).reshape(L, FB, P, D)
        .transpose(0, 2, 1, 3)).astype(f16)

    smallw = np.zeros((L, P, 2 * KB + FB), f32)
    smallw[:, :, 0:KB] = cq.reshape(L, KB, P).transpose(0, 2, 1)
    smallw[:, :, KB:2 * KB] = ck.reshape(L, KB, P).transpose(0, 2, 1)
    smallw[:, :, 2 * KB:] = b1_f.reshape(L, FB, P).transpose(0, 2, 1)

    rows = np.stack([cv, b2], axis=1).astype(f16)  # [L, 2, D]

    # vocab: fold final LN gain/bias, pad each core's shard to 6400
    wout = np.asarray(inputs["Wout"], f32)
    bout = np.asarray(inputs["bout"], f32)
    wout_f = lnfg[:, None] * wout
    bout_f = bout + lnfb @ wout
    VPAD = VC * P * NCORES
    wout_pad = np.zeros((D, VPAD), f32)
    bout_pad = np.zeros((VPAD,), f32)
    for c in range(NCORES):
        lo, hi = c * VS, min((c + 1) * VS, V)
        if lo < V:
            wout_pad[:, c * VC * P:c * VC * P + (hi - lo)] = wout_f[:, lo:hi]
            bout_pad[c * VC * P:c * VC * P + (hi - lo)] = bout_f[lo:hi]

    common = dict(
        emb=emb.astype(f16), posenc=pe, maskadd=maskadd.astype(f16),
        wqkv=wqkv_t, wo=wo_t, w1=w1_t, w2=w2_t,
        smallw=smallw, rows=rows,
    )
    in_maps = []
    for c in range(NCORES):
        m = dict(common)
        m["idx"] = np.ascontiguousarray(idx[c].astype(np.int32).reshape(T, 1))
        ws = wout_pad[:, c * VC * P:(c + 1) * VC * P]  # [D, 6400]
        # wout_h[vc, p, kb*128+n] = ws[kb*128+p, vc*128+n]
        w4 = ws.reshape(KB, P, VC, P).transpose(2, 1, 0, 3).reshape(VC, P, KB * P)
        m["wout"] = np.ascontiguousarray(w4).astype(f16)
        bs = bout_pad[c * VC * P:(c + 1) * VC * P]
        m["bout"] = np.ascontiguousarray(bs.reshape(VC, P).T)
        in_maps.append(m)
    return in_maps


def _unshard(results):
    full = np.zeros((B, T, NCORES * VC * P), np.float32)
    for c in range(NCORES):
        arr = np.asarray(results[c]["logits"], np.float32)  # [VC, TS, P, 512]
        # ts = 2*chunk + b//4; col within ts-block = (b%4)*128 + t_local
        a = arr.reshape(VC, 2, 2, P, 4, P)      # (vc, c2, h2, p, b4, tl)
        a = a.transpose(2, 4, 1, 5, 0, 3)       # (h2, b4, c2, tl, vc, p)
        full[:, :, c * VC * P:(c + 1) * VC * P] = a.reshape(B, T, VC * P)
    out = np.zeros((B, T, V), np.float32)
    for c in range(NCORES):
        lo, hi = c * VS, min((c + 1) * VS, V)
        if lo < V:
            out[:, :, lo:hi] = full[:, :, c * VC * P:c * VC * P + (hi - lo)]
    return out


def kernel(**inputs):
    if "nc" not in _CACHE:
        _CACHE["nc"] = _build_program()
    nc = _CACHE["nc"]
    in_maps = _prep_inputs(inputs)

    if os.environ.get("KERNEL_USE_SIM"):
        from concourse.bass_interp import MultiCoreSim
        sim = MultiCoreSim(nc, num_cores=NCORES,
                           num_workers=int(os.environ.get("KERNEL_SIM_WORKERS", "8")))
        for c in range(NCORES):
            for name, val in in_maps[c].items():
                sim.cores[c].tensor(name)[:] = val
        sim.simulate()
        results = [
            {"logits": np.array(sim.cores[c].tensor("logits"))}
            for c in range(NCORES)
        ]
        return _unshard(results)

    res = bass_utils.run_bass_kernel_spmd(
        nc, in_maps, core_ids=list(range(NCORES)))
    return _unshard(res.results)
